# revision 49
# baseline (speedup 1.0000x reference)
"""TSSA causal self-attention Bass kernel for 8 TRN2 NeuronCores.

Math (per batch b):
    w      = x @ Wa.T + ba                  # (T, C) -> heads (H, T, D)
    wsq    = w * w
    denom  = cumsum_T(wsq)                  # inclusive
    tmp    = (sum_d(wsq / denom) + D*db) * temp          # (T, H)
    Pi     = softmax_h(tmp)                 # (T, H)
    cumA   = cumsum_T(wsq * Pi)
    cumPi  = cumsum_T(Pi) + 1e-8
    y      = -(w * Pi) * (1 / (1 + cumA / cumPi))
           = (w * Pi * cumPi) * (-1 / (cumA + cumPi))
    out    = y @ Wp.T + bp

Sharding: core i -> (batch b = i//2, T-half = i%2).  Each core runs the full
pipeline on its (b, T/2) slice in [t-on-partitions, c-free] layout, chunked by
128 t-rows.  Cumsums over T are triangular matmuls on the PE plus per-chunk
carry rows; the cross-half carries travel via two tiny pairwise AllGathers.
"""

import numpy as np
import ml_dtypes

B, T, C, H, D = 4, 4096, 1024, 16, 64
N_CORES = 8
P = 128
T_LOCAL = T // 2

F32 = None  # filled on bass import
BF16 = None

_BUILD_CACHE = {}



def _ensure_scan_op():
    """Register a custom DVE op: inclusive prefix-scan of Src0*Src1 along the
    free dim.  Per-head sums then come from differencing the scan at head
    boundaries, fusing the (mul, segmented-reduce) pair into one DVE pass."""
    from concourse import dve_ops as dops
    if hasattr(dops, "_TT_MUL_SCAN_ANT"):
        return dops._TT_MUL_SCAN_ANT
    import numpy as np
    from concourse.dve_spec import Spec, Src0, Src1, AluOp, Bin, scan, lower, _has_src1
    from concourse.dve_uop import DveOpSpec

    def _ref(in0, in1, s0, s1, imm2):
        return np.cumsum(in0.astype(np.float32) * in1.astype(np.float32),
                         axis=-1, dtype=np.float32)

    spec = Spec(body=scan(AluOp.ADD, Bin(AluOp.MULTIPLY, Src0, Src1)),
                reference=_ref)
    shas = {}
    for ver in ("v3", "v4"):
        try:
            r = DveOpSpec(name="TT_MUL_SCAN_ANT", uops=lower(spec, ver=ver),
                          rd1_en=_has_src1(spec))
            shas[ver] = r.sha(ver)
        except Exception:
            pass
    op = dops.DveOp("TT_MUL_SCAN_ANT", spec, subdim=False, uops_sha=shas)
    dops.OPS.append(op)
    dops.CUSTOM_DVE_SPECS[op.name] = spec
    dops._SUB_OPCODE_FOR_NAME[op.name] = max(dops._SUB_OPCODE_FOR_NAME.values()) + 1
    dops._TT_MUL_SCAN_ANT = op
    return op

def _build(t_local, n_groups, use_bf16=True, fake_comm=False,
           use_ba=True, use_bp=True, use_tmpscale=True):
    """Build the SPMD Bass program. n_groups = number of core pairs."""
    import concourse.bass as bass
    import concourse.bacc as bacc
    import concourse.mybir as mybir
    from concourse import tile

    dt = mybir.dt
    f32, bf16, f32r = dt.float32, dt.bfloat16, dt.float32r
    AF = mybir.ActivationFunctionType
    OP = mybir.AluOpType

    n_chunks = t_local // P
    n_cores = 2 * n_groups
    NH = n_chunks  # alias

    wdt = bf16 if use_bf16 else f32  # dtype for wsp store + proj2 path
    npwdt = ml_dtypes.bfloat16 if use_bf16 else np.float32

    scan_op = _ensure_scan_op()

    nc = bacc.Bacc(None, target_bir_lowering=False, debug=False)

    # ---------------- I/O ----------------
    xT = nc.dram_tensor("xT", [C, t_local], wdt, kind="ExternalInput")
    waT = nc.dram_tensor("waT", [C, C], wdt, kind="ExternalInput")
    wpTn = nc.dram_tensor("wpTn", [C, C], wdt, kind="ExternalInput")
    ba_in = nc.dram_tensor("ba", [1, C], f32r, kind="ExternalInput")
    onesr_in = nc.dram_tensor("onesr", [1, P], f32r, kind="ExternalInput")
    onesb_in = nc.dram_tensor("onesb", [1, P], wdt, kind="ExternalInput")
    bp_in = nc.dram_tensor("bp", [1, C], wdt, kind="ExternalInput")
    tb_in = nc.dram_tensor("tb", [P, H], f32, kind="ExternalInput")
    db_in = nc.dram_tensor("db64", [t_local, H], f32, kind="ExternalInput")
    pfx_in = nc.dram_tensor("pfx", [NH + 2, NH], f32r, kind="ExternalInput")
    out = nc.dram_tensor("out", [t_local, C], f32, kind="ExternalOutput")

    # constants baked into the NEFF
    ut_np = np.triu(np.ones((P, P), np.float32))
    ut_c = nc.inline_tensor(ut_np, "ut_c")
    utb_c = nc.inline_tensor(ut_np.astype(npwdt), "utb_c")
    ones_np = np.ones((P, P), np.float32)
    ones_c = nc.inline_tensor(ones_np, "ones_c")
    onescol_c = nc.inline_tensor(np.ones((P, 1), np.float32), "onescol_c")
    eye_np = np.eye(P, dtype=np.float32)
    eye_c = nc.inline_tensor(eye_np, "eye_c")
    eyeb_c = nc.inline_tensor(eye_np.astype(npwdt), "eyeb_c")
    bm_np = np.zeros((H, C), np.float32)
    for h in range(H):
        bm_np[h, h * D:(h + 1) * D] = 1.0
    bm_c = nc.inline_tensor(bm_np, "bm_c")
    bmb_c = nc.inline_tensor(bm_np.astype(npwdt), "bmb_c")
    # one-hot chunk selectors: oneh[:, j, m] = (m == j)
    oneh_np = np.zeros((P, NH, NH), np.float32)
    for j in range(NH):
        oneh_np[:, j, j] = 1.0
    oneh_c = nc.inline_tensor(oneh_np.reshape(P, NH * NH), "oneh_c")
    onehb_c = nc.inline_tensor(oneh_np.reshape(P, NH * NH).astype(npwdt),
                               "onehb_c")

    # internal DRAM for collectives
    cc1_in = nc.dram_tensor("cc1_in", [1, C], f32, kind="Internal")
    cc1_out = nc.dram_tensor("cc1_out", [2, C], f32, kind="Internal")
    cc2_in = nc.dram_tensor("cc2_in", [1, C + H], f32, kind="Internal")
    cc2_out = nc.dram_tensor("cc2_out", [2, C + H], f32, kind="Internal")
    rg1 = [[2 * g, 2 * g + 1] for g in range(n_groups)]

    def r(ap):
        return ap.bitcast(f32r)

    with tile.TileContext(nc) as tc:
        with (
            tc.tile_pool(name="const", bufs=1) as cpool,
            tc.tile_pool(name="persist", bufs=1) as pp,
            tc.tile_pool(name="wmat", bufs=1) as wm,
        ):
            # ---- ph1-critical loads first (wa slices, then consts) ----
            wa_t = []
            for a in range(8):
                t = wm.tile([P, C], wdt, tag=f"wa{a}")
                nc.sync.dma_start(
                    t[:, :],
                    waT.ap().rearrange("(a p) n -> a p n", p=P)[a, :, :])
                wa_t.append(t)
            ba_s = cpool.tile([1, C], f32r, tag="ba")
            nc.sync.dma_start(ba_s[:, :], ba_in.ap())
            onesr_s = cpool.tile([1, P], f32r, tag="onesr")
            nc.sync.dma_start(onesr_s[:, :], onesr_in.ap())
            onehb_s = cpool.tile([P, NH, NH], wdt, tag="onehb")
            nc.sync.dma_start(
                onehb_s[:, :, :],
                onehb_c.ap().rearrange("p (j m) -> p j m", j=NH))
            ones_s = cpool.tile([P, P], f32, tag="ones")
            nc.sync.dma_start(ones_s[:, :], ones_c.ap())
            ut_s = cpool.tile([P, P], f32, tag="ut")
            utb_s = cpool.tile([P, P], wdt, tag="utb")
            nc.sync.dma_start(utb_s[:, :], utb_c.ap())
            eye_s = cpool.tile([P, P], f32, tag="eye")
            eyeb_s = cpool.tile([P, P], wdt, tag="eyeb")
            bp_s = cpool.tile([1, C], wdt, tag="bp")
            tb_s = cpool.tile([P, H], f32, tag="tb")
            onesb_s = cpool.tile([1, P], wdt, tag="onesb")
            nc.sync.dma_start(onesb_s[:, :], onesb_in.ap())
            db_s = cpool.tile([P, NH, H], f32, tag="db")
            pfx_s = cpool.tile([NH + 2, NH], f32r, tag="pfx")
            nc.sync.dma_start(pfx_s[:, :], pfx_in.ap())
            onescol_s = cpool.tile([P, 1], f32r, tag="onescol")
            nc.sync.dma_start(onescol_s[:, :], onescol_c.ap().bitcast(f32r))
            oneh_s = cpool.tile([P, NH, NH], f32, tag="oneh")

            def _late_const_loads():
                # ph2/ph3-only constants: issued after ph1 is underway so
                # they don't delay the wa/xt critical path
                nc.sync.dma_start(ut_s[:, :], ut_c.ap())
                nc.sync.dma_start(eye_s[:, :], eye_c.ap())
                nc.sync.dma_start(eyeb_s[:, :], eyeb_c.ap())
                nc.sync.dma_start(bp_s[:, :], bp_in.ap())
                nc.sync.dma_start(tb_s[:, :], tb_in.ap())
                nc.sync.dma_start(db_s[:, :, :],
                                  db_in.ap().rearrange("(j p) h -> p j h", p=P))
                nc.sync.dma_start(
                    oneh_s[:, :, :],
                    oneh_c.ap().rearrange("p (j m) -> p j m", j=NH))

            # ---- persistent stores ----
            w_st = pp.tile([P, NH, C], wdt, tag="w_st")
            sqb_st = pp.tile([P, NH, C], wdt, tag="sqb_st")
            wsp_st = pp.tile([P, NH, C], wdt, tag="wsp_st")
            pi_st = pp.tile([P, NH, H], f32, tag="pi_st")
            bmbx = pp.tile([H + 1, C], wdt, tag="bmbx")
            nc.sync.dma_start(bmbx[0:H, :], bmb_c.ap())
            cpt = pp.tile([H + 1, P], wdt, tag="cpt")
            nc.sync.dma_start(cpt[H:H + 1, :], onesb_in.ap())
            s_tbl = pp.tile([NH + 2, C], f32r, tag="s_tbl")
            swsp_tbl = pp.tile([NH + 2, C], f32r, tag="swsp_tbl")
            spi_tbl = pp.tile([NH + 2, H], f32r, tag="spi_tbl")
            carr_sb = pp.tile([NH, C], wdt, tag="carr_sb")
            carr_fl = pp.tile([1, NH, C], wdt, tag="carr_fl")
            carrA_sb = pp.tile([NH, C], wdt, tag="carrA_sb")
            carrPi_sb = pp.tile([NH, H], f32, tag="carrPi_sb")
            carrPi_fl = pp.tile([1, NH, H], f32, tag="carrPi_fl")

            xT_r = xT.ap().rearrange("(a p) t -> p a t", p=P)

            # ================= phase 1: proj1, store w, S_wsq rows ========
            with (
                tc.tile_pool(name="xt", bufs=3) as xt_pool,
                tc.tile_pool(name="st1", bufs=2) as st_pool,
                tc.tile_pool(name="pw", bufs=2, space="PSUM") as pw_pool,
                tc.tile_pool(name="pS", bufs=1, space="PSUM") as pS_pool,
            ):
                pS = pS_pool.tile([NH, C], f32, tag="pS")
                for j in range(n_chunks):
                    xt = xt_pool.tile([P, 8, P], wdt, tag="xt")
                    nc.scalar.dma_start(xt[:, :, :], xT_r[:, :, j * P:(j + 1) * P])
                    pw = pw_pool.tile([P, C], f32, tag="pw")
                    for hh in range(2):
                        o = pw[:, hh * 512:(hh + 1) * 512]
                        for a in range(8):
                            nc.tensor.matmul(
                                o, xt[:, a, :],
                                wa_t[a][:, hh * 512:(hh + 1) * 512],
                                start=(a == 0),
                                stop=(a == 7 and not use_ba))
                        if use_ba:
                            nc.tensor.matmul(
                                o, onesr_s[0:1, :],
                                ba_s[0:1, hh * 512:(hh + 1) * 512],
                                start=False, stop=True)
                    # drain: w copy (DVE) + square (ACT, persisted for ph2)
                    nc.vector.tensor_copy(w_st[:, j, :], pw[:, :])
                    nc.scalar.activation(sqb_st[:, j, :], pw[:, :], AF.Square)
                    # S_wsq row j (one-hot selector accumulates into row j)
                    for hh in range(2):
                        nc.tensor.matmul(
                            pS[0:NH, hh * 512:(hh + 1) * 512],
                            onehb_s[:, j, :],
                            sqb_st[:, j, hh * 512:(hh + 1) * 512],
                            start=(j == 0), stop=(j == n_chunks - 1))
                _late_const_loads()
                # export S rows + total
                nc.scalar.copy(s_tbl[0:NH, :], pS[:, :])
                pt = pw_pool.tile([P, C], f32, tag="pw")
                for hh in range(2):
                    nc.tensor.matmul(pt[0:1, hh * 512:(hh + 1) * 512],
                                     onescol_s[0:NH, :],
                                     s_tbl[0:NH, hh * 512:(hh + 1) * 512],
                                     start=True, stop=True)
                cc1_stage = st_pool.tile([1, C], f32, tag="cc1s")
                nc.scalar.copy(cc1_stage[:, :], pt[0:1, :])
                nc.sync.dma_start(cc1_in.ap(), cc1_stage[:, :])

            # ============ phase 2: denom, softmax, wsp, S rows ===========
            with (
                tc.tile_pool(name="rd2", bufs=2) as rd_pool,
                tc.tile_pool(name="wn2", bufs=2) as wn_pool,
                tc.tile_pool(name="t2a", bufs=3) as tiny_pool,
                tc.tile_pool(name="st2", bufs=2) as st_pool,
                tc.tile_pool(name="pd", bufs=2, space="PSUM") as pd_pool,
                tc.tile_pool(name="pSP", bufs=1, space="PSUM") as pSP_pool,
                tc.tile_pool(name="pSW", bufs=1, space="PSUM") as pSW_pool,
            ):
                pSPI = pSP_pool.tile([NH, H], f32, tag="pSPI")
                pSW = pSW_pool.tile([NH, C], f32, tag="pSW")

                # prefill: chunk 0/1 local triangulars run during the cc1 wait
                pd_pre = []
                for j in range(2):
                    pd = pd_pool.tile([P, C], f32, tag="pd")
                    for hh in range(2):
                        nc.tensor.matmul(pd[:, hh * 512:(hh + 1) * 512],
                                         utb_s[:, :],
                                         sqb_st[:, j, hh * 512:(hh + 1) * 512],
                                         start=True, stop=False)
                    pd_pre.append(pd)

                # ---- collective 1 + denom carry table ----
                if fake_comm:
                    nc.sync.dma_start(cc1_out.ap()[0:1, :], cc1_in.ap())
                    nc.sync.dma_start(cc1_out.ap()[1:2, :], cc1_in.ap())
                else:
                    nc.gpsimd.collective_compute(
                        "AllGather", mybir.AluOpType.bypass,
                        replica_groups=rg1,
                        ins=[cc1_in.ap().opt()],
                        outs=[cc1_out.ap().opt()])
                nc.sync.dma_start(s_tbl[NH:NH + 2, :],
                                  cc1_out.ap().bitcast(f32r))
                # carry table via the pSW psum (freed by S2's start=True reset)
                for hh in range(2):
                    nc.tensor.matmul(pSW[:, hh * 512:(hh + 1) * 512],
                                     pfx_s[:, :],
                                     s_tbl[:, hh * 512:(hh + 1) * 512],
                                     start=True, stop=True)
                nc.scalar.copy(carr_sb[:, :], pSW[:, :])
                # flatten to one partition so chunk rows read at base 0
                nc.sync.dma_start(carr_fl[0:1, :, :], carr_sb[:, :])

                # prefetch proj2 weights while phase 2 runs
                wp_t = []
                for a in range(8):
                    t = wm.tile([P, C], wdt, tag=f"wa{a}")
                    nc.sync.dma_start(
                        t[:, :],
                        wpTn.ap().rearrange("(a p) n -> a p n", p=P)[a, :, :])
                    wp_t.append(t)

                for j in range(n_chunks):
                    pd = pd_pre[j] if j < 2 else pd_pool.tile([P, C], f32,
                                                              tag="pd")
                    for hh in range(2):
                        o = pd[:, hh * 512:(hh + 1) * 512]
                        if j >= 2:
                            nc.tensor.matmul(
                                o, utb_s[:, :],
                                sqb_st[:, j, hh * 512:(hh + 1) * 512],
                                start=True, stop=False)
                        nc.tensor.matmul(
                            o, onesb_s[0:1, :],
                            carr_fl[0:1, j, hh * 512:(hh + 1) * 512],
                            start=False, stop=True)
                    rd = rd_pool.tile([P, C], f32, tag="rd2")
                    nc.vector.reciprocal_approx_fast(rd[:, :], pd[:, :])
                    wn = wn_pool.tile([P, C], f32, tag="wn2")
                    nc.vector._custom_dve(scan_op, out=wn[:, :],
                                          in0=sqb_st[:, j, :], in1=rd[:, :])
                    wn3 = wn[:, :].rearrange("p (h d) -> p h d", d=D)
                    red = tiny_pool.tile([P, H], f32, tag="red")
                    nc.vector.tensor_copy(red[:, 0:1], wn3[:, 0:1, D - 1])
                    nc.vector.tensor_sub(
                        red[:, 1:H].rearrange("p (h o) -> p h o", o=1),
                        wn3[:, 1:H, D - 1], wn3[:, 0:H - 1, D - 1])
                    if use_tmpscale:
                        t1 = tiny_pool.tile([P, H], f32, tag="t1")
                        nc.vector.tensor_add(t1[:, :], red[:, :], db_s[:, j, :])
                        tmpj = tiny_pool.tile([P, H], f32, tag="tmpj")
                        nc.vector.tensor_mul(tmpj[:, :], t1[:, :], tb_s[:, :])
                    else:
                        tmpj = red
                    # tmp <= D*temp = 64 here, exp(64) fits f32: skip max-sub
                    es = tiny_pool.tile([P, H], f32, tag="es")
                    rs = tiny_pool.tile([P, 1], f32, tag="rs")
                    nc.scalar.activation(es[:, :], tmpj[:, :], AF.Exp,
                                         accum_out=rs[:, :])
                    rr = tiny_pool.tile([P, 1], f32, tag="rr")
                    nc.vector.reciprocal(rr[:, :], rs[:, :])
                    nc.scalar.mul(pi_st[:, j, :], es[:, :], rr[:, :])
                    # last chunk's wsp on DVE (idle by then) so the S2 rows
                    # and cc2 staging aren't stuck behind the Pool queue
                    wsp_eng = (nc.vector if j == n_chunks - 1 else nc.gpsimd)
                    wsp_eng.tensor_mul(
                        wsp_st[:, j, :], sqb_st[:, j, :],
                        pi_st[:, j, :].rearrange("p (h o) -> p h o", o=1)
                        .to_broadcast((P, H, D)))
                    nc.tensor.matmul(pSPI[0:NH, :], oneh_s[:, j, :],
                                     pi_st[:, j, :],
                                     start=(j == 0), stop=(j == n_chunks - 1))
                    for hh in range(2):
                        nc.tensor.matmul(
                            pSW[0:NH, hh * 512:(hh + 1) * 512],
                            onehb_s[:, j, :],
                            wsp_st[:, j, hh * 512:(hh + 1) * 512],
                            start=(j == 0), stop=(j == n_chunks - 1))
                nc.scalar.copy(swsp_tbl[0:NH, :], pSW[:, :])
                nc.scalar.copy(spi_tbl[0:NH, :], pSPI[:, :])
                # totals
                pt2 = pSW_pool.tile([NH, C], f32, tag="pSW")
                for hh in range(2):
                    nc.tensor.matmul(pt2[0:1, hh * 512:(hh + 1) * 512],
                                     onescol_s[0:NH, :],
                                     swsp_tbl[0:NH, hh * 512:(hh + 1) * 512],
                                     start=True, stop=True)
                pt3 = pSP_pool.tile([NH, H], f32, tag="pSPI")
                nc.tensor.matmul(pt3[0:1, 0:H], onescol_s[0:NH, :],
                                 spi_tbl[0:NH, :], start=True, stop=True)
                cc2_stage = st_pool.tile([1, C + H], f32, tag="cc2s")
                nc.scalar.copy(cc2_stage[:, 0:C], pt2[0:1, :])
                nc.scalar.copy(cc2_stage[:, C:C + H], pt3[0:1, 0:H])
                nc.sync.dma_start(cc2_in.ap(), cc2_stage[:, :])

            # ================= phase 3: dots, attn, y, proj2 =============
            with (
                tc.tile_pool(name="t3", bufs=2) as tiny_pool,
                tc.tile_pool(name="rd3", bufs=2) as rd_pool,
                tc.tile_pool(name="yy", bufs=2) as y_pool,
                tc.tile_pool(name="yt", bufs=2) as yt_pool,
                tc.tile_pool(name="ost", bufs=2) as o_pool,
                tc.tile_pool(name="psm", bufs=2, space="PSUM") as psm_pool,
                tc.tile_pool(name="pD", bufs=2, space="PSUM") as pD_pool,
                tc.tile_pool(name="pyt", bufs=2, space="PSUM") as pyt_pool,
                tc.tile_pool(name="po", bufs=2, space="PSUM") as po_pool,
            ):
                # prefill: chunk 0 cumPi/cumA triangulars run during cc2 wait
                ps_pre = psm_pool.tile([P, 512], f32, tag="psm")
                nc.tensor.matmul(ps_pre[:, 0:H], ut_s[:, :], pi_st[:, 0, :],
                                 start=True, stop=False)
                pD_pre = []
                for hh in range(2):
                    pD = pD_pool.tile([P, 512], f32, tag="pD")
                    nc.tensor.matmul(pD[:, :], utb_s[:, :],
                                     wsp_st[:, 0, hh * 512:(hh + 1) * 512],
                                     start=True, stop=False)
                    pD_pre.append(pD)

                # ---- collective 2 + dots carry tables ----
                if fake_comm:
                    nc.sync.dma_start(cc2_out.ap()[0:1, :], cc2_in.ap())
                    nc.sync.dma_start(cc2_out.ap()[1:2, :], cc2_in.ap())
                else:
                    nc.gpsimd.collective_compute(
                        "AllGather", mybir.AluOpType.bypass,
                        replica_groups=rg1,
                        ins=[cc2_in.ap().opt()],
                        outs=[cc2_out.ap().opt()])
                nc.sync.dma_start(swsp_tbl[NH:NH + 2, :],
                                  cc2_out.ap()[:, 0:C].bitcast(f32r))
                nc.sync.dma_start(spi_tbl[NH:NH + 2, :],
                                  cc2_out.ap()[:, C:C + H].bitcast(f32r))
                # carry tables via po psum tiles (freed before proj2 starts)
                for hh in range(2):
                    poc = po_pool.tile([P, 512], f32, tag="po")
                    nc.tensor.matmul(poc[0:NH, :], pfx_s[:, :],
                                     swsp_tbl[:, hh * 512:(hh + 1) * 512],
                                     start=True, stop=True)
                    # one half per engine so the copies run in parallel
                    eng = nc.scalar if hh == 0 else nc.vector
                    if hh == 0:
                        nc.scalar.copy(carrA_sb[:, hh * 512:(hh + 1) * 512],
                                       poc[0:NH, :])
                    else:
                        nc.vector.tensor_copy(
                            carrA_sb[:, hh * 512:(hh + 1) * 512],
                            poc[0:NH, :])
                poc3 = po_pool.tile([P, 512], f32, tag="po")
                nc.tensor.matmul(poc3[0:NH, 0:H], pfx_s[:, :], spi_tbl[:, :],
                                 start=True, stop=True)
                nc.scalar.copy(carrPi_sb[:, :], poc3[0:NH, 0:H])
                nc.sync.dma_start(carrPi_fl[0:1, :, :], carrPi_sb[:, :])

                for j in range(n_chunks):
                    # cumPi for this chunk
                    ps = ps_pre if j == 0 else psm_pool.tile([P, 512], f32,
                                                             tag="psm")
                    if j > 0:
                        nc.tensor.matmul(ps[:, 0:H], ut_s[:, :],
                                         pi_st[:, j, :],
                                         start=True, stop=False)
                    nc.tensor.matmul(ps[:, 0:H], ones_s[0:1, :],
                                     carrPi_fl[0:1, j, :],
                                     start=False, stop=True)
                    cpe = tiny_pool.tile([P, H], f32, tag="cpe")
                    nc.vector.tensor_scalar_add(cpe[:, :], ps[:, 0:H], 1e-8)
                    # transpose cpe -> [H, P]; stationary [H+1, P] also holds
                    # a ones row so the carrA add rides the same matmul
                    nc.tensor.transpose(ps[0:H, 128:256], cpe[:, :],
                                        eye_s[:, :])
                    nc.vector.tensor_copy(cpt[0:H, :], ps[0:H, 128:256])
                    # carrA row for this chunk under the bm selector rows
                    nc.scalar.dma_start(bmbx[H:H + 1, :], carrA_sb[j:j + 1, :])
                    # D = cumA + bcast(cumPi_e)  (carry + bcast in one matmul)
                    rd = rd_pool.tile([P, C], f32, tag="rd3")
                    for hh in range(2):
                        pD = pD_pre[hh] if j == 0 else pD_pool.tile(
                            [P, 512], f32, tag="pD")
                        if j > 0:
                            nc.tensor.matmul(
                                pD[:, :], utb_s[:, :],
                                wsp_st[:, j, hh * 512:(hh + 1) * 512],
                                start=True, stop=False)
                        nc.tensor.matmul(pD[:, :], cpt[:, :],
                                         bmbx[:, hh * 512:(hh + 1) * 512],
                                         start=False, stop=True)
                        nc.vector.reciprocal_approx_fast(
                            rd[:, hh * 512:(hh + 1) * 512], pD[:, :])
                    # g = Pi * cumPi_e
                    g = tiny_pool.tile([P, H], f32, tag="g")
                    nc.vector.tensor_mul(g[:, :], pi_st[:, j, :], cpe[:, :])
                    # y = (w * rd) * g_bcast   (positive; sign folded into wpTn)
                    t1 = y_pool.tile([P, C], f32, tag="t1f")
                    nc.vector.tensor_mul(t1[:, :], w_st[:, j, :], rd[:, :])
                    y = y_pool.tile([P, C], wdt, tag="ybf")
                    nc.vector.tensor_mul(
                        y[:, :], t1[:, :],
                        g[:, :].rearrange("p (h o) -> p h o", o=1).to_broadcast((P, H, D)))
                    # transpose y -> yT (8 PE transposes via 2 psum rounds)
                    yt = yt_pool.tile([P, 8, P], wdt, tag="yt")
                    for rnd in range(2):
                        pyt = pyt_pool.tile([P, 512], wdt, tag="pyt")
                        for i in range(4):
                            a = rnd * 4 + i
                            nc.tensor.transpose(
                                pyt[:, i * P:(i + 1) * P],
                                y[:, a * P:(a + 1) * P], eyeb_s[:, :])
                        nc.scalar.copy(yt[:, rnd * 4:rnd * 4 + 4, :],
                                       pyt[:, :])
                    # proj2
                    for hh in range(2):
                        po = po_pool.tile([P, 512], f32, tag="po")
                        for a in range(8):
                            nc.tensor.matmul(
                                po[:, :], yt[:, a, :],
                                wp_t[a][:, hh * 512:(hh + 1) * 512],
                                start=(a == 0),
                                stop=(a == 7 and not use_bp))
                        if use_bp:
                            nc.tensor.matmul(po[:, :], onesb_s[0:1, :],
                                             bp_s[0:1, hh * 512:(hh + 1) * 512],
                                             start=False, stop=True)
                        ost = o_pool.tile([P, 512], f32, tag="ost")
                        nc.scalar.copy(ost[:, :], po[:, :])
                        nc.sync.dma_start(
                            out.ap()[j * P:(j + 1) * P,
                                     hh * 512:(hh + 1) * 512], ost[:, :])

    nc.finalize()
    return nc


def _get_nc(t_local=T_LOCAL, n_groups=4, use_bf16=True, fake_comm=False,
            use_ba=True, use_bp=True, use_tmpscale=True):
    key = (t_local, n_groups, use_bf16, fake_comm, use_ba, use_bp, use_tmpscale)
    if key not in _BUILD_CACHE:
        _BUILD_CACHE[key] = _build(t_local, n_groups, use_bf16, fake_comm,
                                   use_ba, use_bp, use_tmpscale)
    return _BUILD_CACHE[key]


def make_in_maps(x, Wa, ba, Wp, bp, temp, denom_bias, t_local=T_LOCAL,
                 n_groups=4, use_bf16=True):
    """Host-side sharding: core i -> (b=i//2, half=i%2)."""
    npwdt = ml_dtypes.bfloat16 if use_bf16 else np.float32
    n_chunks = t_local // P
    waT = np.ascontiguousarray(Wa.T).astype(npwdt)
    onesr = np.ones((1, P), np.float32)
    onesb = np.ones((1, P), npwdt)
    wpTn = np.ascontiguousarray((-Wp.T).astype(np.float32)).astype(npwdt)
    ba_r = np.ascontiguousarray(ba.reshape(1, C).astype(np.float32))
    bp_r = np.ascontiguousarray(bp.reshape(1, C).astype(np.float32)).astype(npwdt)
    tb = np.broadcast_to(temp.reshape(1, H), (P, H)).astype(np.float32)
    tb = np.ascontiguousarray(tb)
    in_maps = []
    for i in range(2 * n_groups):
        b, half = i // 2, i % 2
        t0 = half * t_local
        xT = np.ascontiguousarray(x[b, t0:t0 + t_local, :].T).astype(npwdt)
        db64 = np.ascontiguousarray(
            (D * denom_bias[:, t0:t0 + t_local, 0].T).astype(np.float32))
        pfx = np.zeros((n_chunks + 2, n_chunks), np.float32)
        for k in range(n_chunks):
            pfx[k, k + 1:] = 1.0
        if half == 1:
            pfx[n_chunks, :] = 1.0  # partner (pair rank 0) total
        in_maps.append({
            "xT": xT, "waT": waT, "wpTn": wpTn, "ba": ba_r, "bp": bp_r,
            "tb": tb, "db64": db64, "pfx": pfx, "onesr": onesr, "onesb": onesb,
        })
    return in_maps


def kernel(x, Wa, ba, Wp, bp, temp, denom_bias):
    x = np.asarray(x)
    use_ba = bool(np.any(np.asarray(ba)))
    use_bp = bool(np.any(np.asarray(bp)))
    use_tmpscale = bool(np.any(np.asarray(denom_bias))
                        or not np.all(np.asarray(temp) == 1.0))
    nc = _get_nc(use_ba=use_ba, use_bp=use_bp, use_tmpscale=use_tmpscale)
    in_maps = make_in_maps(np.asarray(x), np.asarray(Wa), np.asarray(ba),
                           np.asarray(Wp), np.asarray(bp), np.asarray(temp),
                           np.asarray(denom_bias))
    from concourse import bass_utils
    res = bass_utils.run_bass_kernel_spmd(nc, in_maps, core_ids=list(range(N_CORES)))
    out = np.empty((B, T, C), np.float32)
    for i in range(N_CORES):
        b, half = i // 2, i % 2
        out[b, half * T_LOCAL:(half + 1) * T_LOCAL, :] = res.results[i]["out"]
    return out

